# revision 47
# baseline (speedup 1.0000x reference)
"""Trainium2 Bass kernel for nn_AttentionOnDetail (sparse patch attention).

Data-parallel over batch B=8 across 8 NeuronCores; one batch per core.

v2 design (latency-focused; the kernel is dependency-bound, not
throughput-bound):
  - Host-side prep inside kernel(): W_qkvg.T / W_out.T passed
    pre-transposed, cos/sin tables pre-permuted and duplicated,
    rmsnorm(sink)*tao rows precomputed (rope at position 0 is identity).
  - x tile DMAs issued first; patch stats (ACT square+accum, DVE dot)
    pipeline behind them; per-tile logits transposed into a PSUM row via
    PE so top-4 selection needs no DMA.
  - Top-4 via max8/threshold/max_index (ascending patch order); token
    gather via one indirect DMA of 4 whole patches.
  - qkvg projection computed "PE-direct": per (tensor, kilo-block)
    matmuls with column-selected lhsT views and stride-4 PSUM output
    rows land q/k/v/g directly in attention layout (no DRAM bounce, no
    rearrange DMAs).  fp32r operands -> 1 cycle/row.
  - q and k stacked on 128 partitions: rmsnorm+rope for both costs one
    set of full-width DVE ops (cost scales with free size only).
    rsqrt via ln+exp keeps ACT on a single function table; tao folded
    into the exp bias.
  - attention: bf16 matmuls, no row-max softmax (range is bounded),
    one exp over all heads, 1/den folded into p before the transpose.
"""

import sys
import numpy as np

for _p in ("/opt/trn_rl_repo",):
    if _p not in sys.path:
        sys.path.insert(0, _p)

import concourse.bass as bass
import concourse.bacc as bacc
import concourse.tile as tile
from concourse import mybir
from concourse.bass_utils import run_bass_kernel_spmd

F32 = mybir.dt.float32
F32R = mybir.dt.float32r
BF16 = mybir.dt.bfloat16
I32 = mybir.dt.int32
U32 = mybir.dt.uint32
U16 = mybir.dt.uint16
AF = mybir.ActivationFunctionType
ALU = mybir.AluOpType
AX = mybir.AxisListType

B, T, C, H, T0 = 8, 8192, 128, 8, 16
NP = T // T0          # 512 patches
PATCH = T0 * C        # 2048 elements per patch
S = 65                # sink + 64 selected tokens
NSEL = 64
EPS = 1.1920929e-07
SCALE = 1.0 / float(np.sqrt(np.float32(C)))
NEG_BIG = -1.0e30


def rap(t, apl, offset=0):
    """Raw AP over a tile/AP's storage, flat element strides."""
    base = t if isinstance(t, bass.AP) else t[:]
    return bass.AP(tensor=base.tensor, offset=base.offset + offset,
                   ap=[list(x) for x in apl])


def f32r(ap):
    return ap.bitcast(F32R)


def build_kernel(nc):
    xb = nc.dram_tensor("xb", [T, C], F32, kind="ExternalInput")
    pw = nc.dram_tensor("pw", [1, PATCH + 128], F32R, kind="ExternalInput")
    wqT_d = nc.dram_tensor("wqT_d", [C, 4 * C * H], F32R, kind="ExternalInput")
    woT_d = nc.dram_tensor("woT_d", [C, H, C], F32, kind="ExternalInput")
    tabs = nc.dram_tensor("tabs", [128, 401], F32, kind="ExternalInput")
    negio = nc.dram_tensor("negio", [1, NP], F32, kind="ExternalInput")
    sinkvb = nc.dram_tensor("sinkvb", [1, H * C], U16, kind="ExternalInput")
    ident = nc.dram_tensor("ident", [128, 128], F32, kind="ExternalInput")
    out = nc.dram_tensor("out", [NSEL, C], F32, kind="ExternalOutput")

    with tile.TileContext(nc) as tc:
        _emit(tc, nc, xb, pw, wqT_d, woT_d, tabs,
              negio, sinkvb, ident, out)
    return nc


def _emit(tc, nc, xb, pw, wqT_d, woT_d, tabs,
          negio, sinkvb, ident, out):
    import os
    LEVEL = int(os.environ.get("KLEVEL", "9"))
    from contextlib import ExitStack
    ctx = ExitStack()
    with ctx:
        const1 = ctx.enter_context(tc.tile_pool(name="const1", bufs=1))
        xpool = ctx.enter_context(tc.tile_pool(name="xpool", bufs=1))
        junkp = ctx.enter_context(tc.tile_pool(name="junkp", bufs=1))
        stat = ctx.enter_context(tc.tile_pool(name="stat", bufs=4))
        sb = ctx.enter_context(tc.tile_pool(name="sb", bufs=1))
        psall = ctx.enter_context(tc.tile_pool(name="psall", bufs=1,
                                               space="PSUM"))
        # one tile owning all 8 PSUM banks; regions choreographed manually
        PS = psall.tile([128, 4096], F32)
        # region plan (f32 cols):
        #   0:1024     qk stack (q rows 0:64, k rows 64:128); later att
        #              [65, 520] at 0:520; later out [64, 128]
        #   1024:2048  v rows 0:64; later pT staging [65, 260] at 1024:1284
        #   2048:3072  g rows 0:64
        #   2560:3584  y [65, 1024] (after sigg consumed g)
        #   3072:3584  staging A (pw bcast, x_selT, qnT)
        #   3584:4096  logits row (rows 0:1) / knT staging / ygT staging
        LROW = 3584

        # ---------------- pw/ones + ident first, then the x stream --------
        pwo_sb = const1.tile([1, PATCH + 128], F32R)
        nc.sync.dma_start(out=pwo_sb[:, :], in_=pw[:, :])
        pw_sb = pwo_sb[0:1, 0:PATCH]
        ones_t = pwo_sb[0:1, PATCH:PATCH + 128]
        ident_t = const1.tile([128, 128], F32)
        nc.sync.dma_start(out=ident_t[:, :], in_=ident[:, :])

        def xdma(i):
            xp = xpool.tile([128, PATCH], F32, tag=f"xp{i}")
            nc.sync.dma_start(
                out=xp[:, :],
                in_=rap(xb[:, :], [[PATCH, 128], [1, PATCH]],
                        offset=i * 128 * PATCH))
            return xp

        xps = [xdma(0)]
        XSPLIT = True
        eps_t = const1.tile([128, 1], F32)
        nc.vector.memset(eps_t[:, :], EPS)

        # ---------------- remaining x tiles (tile 3 in halves) ----------
        for i in (1, 2):
            xps.append(xdma(i))
        xp3 = xpool.tile([128, PATCH], F32, tag="xp3")
        for hh in range(2):
            nc.sync.dma_start(
                out=xp3[:, 1024 * hh:1024 * (hh + 1)],
                in_=rap(xb[:, :], [[PATCH, 128], [1, 1024]],
                        offset=3 * 128 * PATCH + 1024 * hh))
        xps.append(xp3)
        tabs_t = const1.tile([128, 401], F32)
        nc.sync.dma_start(out=tabs_t[:, :], in_=tabs[:, :])
        cosdup_t = tabs_t[:, 0:128]
        sinpm_t = tabs_t[:, 128:256]
        sinkTq_t = tabs_t[:, 256:264]
        sinkTk_t = tabs_t[:, 264:272]
        cmask_t = tabs_t[0:S, 272:337]
        sel16_t = tabs_t[0:5, 337:401]
        negio_t = const1.tile([1, NP], F32)
        nc.sync.dma_start(out=negio_t[:, :], in_=negio[:, :])
        rhs5 = const1.tile([5, 1], F32)
        nc.vector.memset(rhs5[:, :], 1.0)

        # v sink row: host-rounded bf16 bits straight into v_sb row 64
        v_sb = sb.tile([S, H, C], BF16, tag="v_sb")
        nc.sync.dma_start(
            out=v_sb[NSEL:S, :, :],
            in_=sinkvb[:, :].bitcast(BF16).rearrange(
                "p (h c) -> p h c", h=H))

        wqT = const1.tile([C, 4 * C * H], F32R)
        for wch in range(4):
            nc.sync.dma_start(out=wqT[:, 1024 * wch:1024 * (wch + 1)],
                              in_=wqT_d[:, 1024 * wch:1024 * (wch + 1)])
        woT = const1.tile([C, H, C], F32)


        # preload the sqrt activation table while ACT is idle
        dummy = stat.tile([1, 1], F32)
        nc.vector.memset(dummy[:, :], 1.0)
        nc.scalar.activation(out=dummy[:, :], in_=dummy[:, :],
                             func=AF.Sqrt)

        woTb = const1.tile([C, H, C], BF16)

        # pw broadcast to 128 partitions via K=1 matmul into staging banks
        # (two tiny warmups first lift PE off the cold p-state)
        nc.tensor.matmul(out=PS[0:128, 3071:3072], lhsT=ident_t[:, :],
                         rhs=ident_t[:, 0:1], start=True, stop=True)
        nc.tensor.matmul(out=PS[0:128, 3071:3072], lhsT=ident_t[:, :],
                         rhs=ident_t[:, 0:1], start=True, stop=True)
        pwB = const1.tile([128, PATCH], F32)
        for q4 in range(4):
            base = 1536 + 512 * q4
            nc.tensor.matmul(out=PS[:, base:base + 512],
                             lhsT=ones_t,
                             rhs=pwo_sb[0:1, 512 * q4:512 * (q4 + 1)],
                             start=True, stop=True)
            if q4 % 2 == 0:
                nc.scalar.copy(out=pwB[:, 512 * q4:512 * (q4 + 1)],
                               in_=PS[:, base:base + 512])
            else:
                nc.vector.tensor_copy(out=pwB[:, 512 * q4:512 * (q4 + 1)],
                                      in_=PS[:, base:base + 512])

        # ---------------- phase 1: per-patch stats ----------------
        junk = junkp.tile([128, PATCH], F32, tag="junk")
        junk2 = junkp.tile([128, PATCH], F32, tag="junk2")
        ss_c = stat.tile([128, 4], F32, tag="ss_c")
        dot_c = stat.tile([128, 4], F32, tag="dot_c")
        rs_c = stat.tile([128, 4], F32, tag="rs_c")
        logit_c = stat.tile([128, 4], F32, tag="logit_c")
        ss_h = stat.tile([128, 2], F32, tag="ss_h")
        dot_h = stat.tile([128, 2], F32, tag="dot_h")
        for i in range(4):
            xp = xps[i]
            if i < 3:
                nc.scalar.activation(out=junk[:, :], in_=xp[:, :],
                                     func=AF.Square,
                                     accum_out=ss_c[:, i:i + 1])
                nc.vector.scalar_tensor_tensor(
                    out=junk2[:, :], in0=xp[:, :], scalar=1.0, in1=pwB[:, :],
                    op0=ALU.mult, op1=ALU.mult,
                    accum_out=dot_c[:, i:i + 1])
            else:
                # tile 3 in halves so its stats overlap its own DMA
                for hh in range(2):
                    cs = slice(1024 * hh, 1024 * (hh + 1))
                    nc.scalar.activation(out=junk[:, cs], in_=xp[:, cs],
                                         func=AF.Square,
                                         accum_out=ss_h[:, hh:hh + 1])
                    nc.vector.scalar_tensor_tensor(
                        out=junk2[:, cs], in0=xp[:, cs], scalar=1.0,
                        in1=pwB[:, cs], op0=ALU.mult, op1=ALU.mult,
                        accum_out=dot_h[:, hh:hh + 1])
                nc.scalar.activation(out=ss_c[:, 3:4], in_=ss_h[:, 0:1],
                                     func=AF.Identity, bias=ss_h[:, 1:2])
                nc.vector.tensor_add(out=dot_c[:, 3:4], in0=dot_h[:, 0:1],
                                     in1=dot_h[:, 1:2])
            nc.scalar.activation(out=rs_c[:, i:i + 1], in_=ss_c[:, i:i + 1],
                                 func=AF.Sqrt, bias=eps_t[:, :],
                                 scale=1.0 / PATCH)
            nc.vector.reciprocal(out=rs_c[:, i:i + 1], in_=rs_c[:, i:i + 1])
            nc.vector.tensor_mul(logit_c[:, i:i + 1], dot_c[:, i:i + 1],
                                 rs_c[:, i:i + 1])
            # transpose this tile's logits column into the PSUM row
            nc.tensor.transpose(
                out=PS[0:1, LROW + 128 * i:LROW + 128 * (i + 1)],
                in_=logit_c[:, i:i + 1], identity=ident_t[:, :])

        if LEVEL == 1:
            lrow_sb = stat.tile([1, NP], F32, tag="lrow_sb")
            nc.vector.tensor_copy(out=lrow_sb[:, :],
                                  in_=PS[0:1, LROW:LROW + NP])
            for r in range(4):
                nc.sync.dma_start(out=out[r:r + 1, :],
                                  in_=lrow_sb[0:1, 128 * r:128 * (r + 1)])
            return

        # ---------------- top-4 selection (on the PSUM row) ----------------
        lrow = PS[0:1, LROW:LROW + NP]
        max8 = stat.tile([1, 8], F32, tag="max8")
        nc.vector.max(out=max8[:, :], in_=lrow)
        masked = stat.tile([1, NP], F32, tag="masked")
        nc.vector.scalar_tensor_tensor(
            out=masked[:, :], in0=lrow, scalar=max8[:, 3:4],
            in1=negio_t[:, :], op0=ALU.is_ge, op1=ALU.mult)
        mm8 = stat.tile([1, 8], F32, tag="mm8")
        nc.vector.max(out=mm8[:, :], in_=masked[:, :])
        idx8 = stat.tile([1, 8], U32, tag="idx8")
        nc.vector.max_index(out=idx8[:, :], in_max=mm8[:, :],
                            in_values=masked[:, :])
        idxf = stat.tile([1, 8], F32, tag="idxf")
        nc.vector.tensor_copy(out=idxf[:, :], in_=idx8[:, :])

        # patch-id column via PE transpose: [1,4] -> [4,1], then token
        # ids 16*I[p] + 4T + t in (T, p, t) row order via sel16
        nc.tensor.transpose(out=PS[0:4, 3582:3583], in_=idxf[0:1, 0:4],
                            identity=ident_t[0:1, 0:1])
        nc.scalar.copy(out=rhs5[0:4, :], in_=PS[0:4, 3582:3583])
        nc.tensor.matmul(out=PS[0:NSEL, 3583:3584], lhsT=sel16_t[:, :],
                         rhs=rhs5[:, :], start=True, stop=True)
        idc_i = stat.tile([NSEL, 1], I32, tag="idc_i")
        nc.vector.tensor_copy(out=idc_i[:, :], in_=PS[0:NSEL, 3583:3584])

        # gather the 64 tokens (row 16T+4p+t = token 16*I[p] + 4T + t)
        x_sel = sb.tile([NSEL, C], F32, tag="x_sel")
        nc.gpsimd.indirect_dma_start(
            out=x_sel[:, :], out_offset=None, in_=xb[:, :],
            in_offset=bass.IndirectOffsetOnAxis(ap=idc_i[:, 0:1], axis=0))


        if LEVEL == 2:
            nc.sync.dma_start(out=out[:, :], in_=x_sel[:, :])
            return

        # ---------------- qkvg projection ----------------
        nc.tensor.transpose(out=PS[0:128, 3072:3072 + NSEL], in_=x_sel[:, :],
                            identity=ident_t[0:NSEL, 0:NSEL])
        x_selT = sb.tile([C, NSEL], F32R, tag="x_selT")
        nc.scalar.copy(out=x_selT[:, :], in_=PS[:, 3072:3072 + NSEL])

        # qkvg[token, f] for the 64 gathered tokens -> PS rows 0:64
        for g in range(8):
            nc.tensor.matmul(out=PS[0:NSEL, 512 * g:512 * (g + 1)],
                             lhsT=x_selT[:, :],
                             rhs=wqT[:, 512 * g:512 * (g + 1)],
                             start=True, stop=True)
        qkvg_sb = sb.tile([NSEL, 4 * C * H], BF16, tag="qkvg_sb")
        nc.scalar.copy(out=qkvg_sb[:, 0:1024], in_=PS[0:NSEL, 0:1024])
        nc.vector.tensor_copy(out=qkvg_sb[:, 1024:2048],
                              in_=PS[0:NSEL, 1024:2048])
        nc.scalar.copy(out=qkvg_sb[:, 2048:3072], in_=PS[0:NSEL, 2048:3072])
        nc.vector.tensor_copy(out=qkvg_sb[:, 3072:4096],
                              in_=PS[0:NSEL, 3072:4096])

        # rearrange token-major -> s-major via SBUF->SBUF DMAs.
        # qkvg row 16T+4p+t (token 16*I[p]+4T+t), col (b,h,c) feeds
        # s-row 16p+4t+b of tensor T: per tensor the source rows are the
        # contiguous block 16T:16T+16 -> clean single-stride APs.
        qk = sb.tile([128, H, C], BF16, tag="qk")
        vg = sb.tile([128, H, C], BF16, tag="vg")
        FQ = 4 * C * H

        def rearr(tens, dst, half, eng):
            eng.dma_start(
                out=dst[64 * half:64 * half + NSEL, :, :],
                in_=rap(qkvg_sb[:, :], [[FQ, T0], [1024, 4], [1, 1024]],
                        offset=T0 * tens * FQ))

        rearr(0, qk, 0, nc.sync)    # q on the SP queue
        rearr(1, qk, 1, nc.scalar)  # k on the ACT queue (packs transfers)

        # out-projection weights arrive late; the dummy write makes the DMA
        # wait for the gather so it cannot block the gather's transfer
        nc.vector.tensor_copy(out=woT[0:1, 0, 0:1], in_=x_sel[0:1, 0:1])
        nc.sync.dma_start(out=woT[:, :, :], in_=woT_d[:, :, :])
        nc.gpsimd.tensor_copy(out=woTb[:, :, :], in_=woT[:, :, :])

        if LEVEL == 3:
            q0 = sb.tile([NSEL, C], F32, tag="q0dbg")
            nc.vector.tensor_copy(out=q0[:, :], in_=qk[0:NSEL, 0, :])
            nc.sync.dma_start(out=out[:, :], in_=q0[:, :])
            return

        # ---------------- rmsnorm + rope on the qk stack ----------------
        ssq = sb.tile([128, H], F32, tag="ssq")
        sqj = junkp.tile([128, H, C], F32, tag="sqj")
        nc.gpsimd.tensor_tensor(out=sqj[:, 5:8, :], in0=qk[:, 5:8, :],
                                in1=qk[:, 5:8, :], op=ALU.mult)
        nc.vector.tensor_tensor(out=sqj[:, 0:5, :], in0=qk[:, 0:5, :],
                                in1=qk[:, 0:5, :], op=ALU.mult)
        # gate the v/g rearranges on the stt output so the scheduler keeps
        # sigmoid's table load behind the rope sqrt (value-preserving
        # corner write on the v/g source rows)
        zro = stat.tile([17, 1], F32, tag="zro")
        nc.vector.tensor_scalar_mul(zro[:, :], sqj[0:17, 0, 0:1], 0.0)
        corner = rap(qkvg_sb[:, :], [[FQ, 17], [1, 1]], offset=32 * FQ)
        nc.vector.tensor_scalar(out=corner, in0=corner,
                                scalar1=zro[:, 0:1], scalar2=None,
                                op0=ALU.add)
        rearr(2, vg, 0, nc.sync)
        rearr(3, vg, 1, nc.sync)
        nc.gpsimd.tensor_copy(out=v_sb[0:NSEL, :, :], in_=vg[0:NSEL, :, :])
        nc.vector.tensor_reduce(out=ssq[:, 0:5], in_=sqj[:, 0:5, :],
                                axis=AX.X, op=ALU.add)
        nc.vector.tensor_reduce(out=ssq[:, 5:8], in_=sqj[:, 5:8, :],
                                axis=AX.X, op=ALU.add)
        rf = sb.tile([128, H], F32, tag="rf")
        nc.scalar.activation(out=rf[:, :], in_=ssq[:, :], func=AF.Sqrt,
                             bias=eps_t[:, :], scale=1.0 / C)
        nc.vector.reciprocal(out=rf[:, :], in_=rf[:, :])
        # sigmoid gate now, exp loads after: both ACT table loads land in
        # the rope shadow, and the softmax exps then run load-free
        sigg = sb.tile([NSEL, H, C], BF16, tag="sigg")
        nc.scalar.activation(out=sigg[:, :, :], in_=vg[NSEL:128, :, :],
                             func=AF.Sigmoid)
        qk1 = sb.tile([128, H, C], F32, tag="qk1")
        r1 = sb.tile([128, H, C], F32, tag="r1")
        r2 = sb.tile([128, H, C], F32, tag="r2")
        qkn = sb.tile([128, H, C], F32, tag="qkn")

        def hs(eng, hs0, hs1):
            hn = hs1 - hs0
            eng.tensor_tensor(
                out=qk1[:, hs0:hs1, :], in0=qk[:, hs0:hs1, :],
                in1=rf[:, hs0:hs1].rearrange("p (h a) -> p h a", a=1)
                    .to_broadcast([128, hn, C]), op=ALU.mult)
            eng.tensor_tensor(
                out=r1[:, hs0:hs1, :], in0=qk1[:, hs0:hs1, :],
                in1=cosdup_t[:, :].rearrange("p (a c) -> p a c", a=1)
                    .to_broadcast([128, hn, C]), op=ALU.mult)
            eng.tensor_tensor(
                out=r2[:, hs0:hs1, 0:64], in0=qk1[:, hs0:hs1, 64:128],
                in1=sinpm_t[:, 0:64].rearrange("p (a c) -> p a c", a=1)
                    .to_broadcast([128, hn, 64]), op=ALU.mult)
            eng.tensor_tensor(
                out=r2[:, hs0:hs1, 64:128], in0=qk1[:, hs0:hs1, 0:64],
                in1=sinpm_t[:, 64:128].rearrange("p (a c) -> p a c", a=1)
                    .to_broadcast([128, hn, 64]), op=ALU.mult)
            eng.tensor_add(out=qkn[:, hs0:hs1, :], in0=r1[:, hs0:hs1, :],
                           in1=r2[:, hs0:hs1, :])

        hs(nc.vector, 0, 5)
        hs(nc.gpsimd, 5, 8)

        if LEVEL == 4:
            qn32 = sb.tile([NSEL, C], F32, tag="qn32")
            nc.vector.tensor_copy(out=qn32[:, :], in_=qkn[0:NSEL, 0, :])
            nc.sync.dma_start(out=out[:, :], in_=qn32[:, :])
            return

        # ---------------- transposes to qnT / knT ----------------
        # per head-group so group-0 attention starts while group-1 is
        # still transposing; sink columns inserted up front
        qnT = sb.tile([C, H, S], BF16, tag="qnT")
        knT = sb.tile([C, H, S], BF16, tag="knT")
        nc.scalar.copy(out=rap(qnT[:, :, :], [[H * S, C], [S, H], [1, 1]],
                               offset=NSEL),
                       in_=sinkTq_t[:, :].rearrange("c (h a) -> c h a", a=1))
        nc.scalar.copy(out=rap(knT[:, :, :], [[H * S, C], [S, H], [1, 1]],
                               offset=NSEL),
                       in_=sinkTk_t[:, :].rearrange("c (h a) -> c h a", a=1))
        for g in range(2):
            for si, dstT in enumerate((qnT, knT)):
                base = 3072 + 256 * (2 * g + si)
                for j in range(4):
                    h = 4 * g + j
                    nc.tensor.transpose(
                        out=PS[:, base + NSEL * j:base + NSEL * (j + 1)],
                        in_=qkn[64 * si:64 * (si + 1), h, :],
                        identity=ident_t[64 * si:64 * si + NSEL,
                                         64 * si:64 * si + NSEL])
                dst = rap(dstT[:, :, :], [[H * S, C], [S, 4], [1, NSEL]],
                          offset=4 * g * S)
                nc.vector.tensor_copy(
                    out=dst, in_=PS[:, base:base + 256].rearrange(
                        "p (h s) -> p h s", h=4))

        # ---------------- attention ----------------
        # att head slots padded to 128 cols (matmul must not cross banks);
        # the whole tail runs as two independent head-group pipelines so
        # PE/DVE/ACT overlap across groups
        t0 = sb.tile([S, H, S], F32, tag="t0")
        p_sb = sb.tile([S, H, S], F32, tag="p_sb")
        den8 = sb.tile([S, H], F32, tag="den8")
        rden = sb.tile([S, H], F32, tag="rden")
        sigrd = sb.tile([NSEL, H, C], F32, tag="sigrd")
        pT = sb.tile([S, H, S], BF16, tag="pT")
        ygT = sb.tile([C, H, NSEL], BF16, tag="ygT")
        sgT_sb = sb.tile([C, H, NSEL], BF16, tag="sgT_sb")
        for g in range(2):
            hs = slice(4 * g, 4 * (g + 1))
            for h in range(4 * g, 4 * (g + 1)):
                nc.tensor.matmul(out=PS[0:S, C * h:C * h + S],
                                 lhsT=qnT[:, h, :], rhs=knT[:, h, :],
                                 start=True, stop=True)
            attg = rap(PS[:, :], [[4096, S], [C, 4], [1, S]],
                       offset=4 * g * C)
            nc.vector.tensor_tensor(
                out=t0[:, hs, :], in0=attg,
                in1=cmask_t[:, :].rearrange("s (a t) -> s a t", a=1)
                    .to_broadcast([S, 4, S]), op=ALU.add)
            nc.scalar.activation(out=p_sb[:, hs, :], in_=t0[:, hs, :],
                                 func=AF.Exp, scale=SCALE)
            nc.vector.tensor_reduce(out=den8[:, hs], in_=p_sb[:, hs, :],
                                    axis=AX.X, op=ALU.add)
            nc.vector.reciprocal(out=rden[:, hs], in_=den8[:, hs])
            # 1/den folds into the gate; pT/y consume UNNORMALIZED p
            nc.vector.tensor_tensor(
                out=sigrd[:, hs, :], in0=sigg[:, hs, :],
                in1=rden[0:NSEL, hs].rearrange("s (h a) -> s h a", a=1)
                    .to_broadcast([NSEL, 4, C]), op=ALU.mult)
            for j in range(4):
                nc.tensor.transpose(
                    out=PS[0:S, 1024 + 520 * g + S * j:
                           1024 + 520 * g + S * (j + 1)],
                    in_=p_sb[:, 4 * g + j, :], identity=ident_t[0:S, 0:S])
            nc.scalar.copy(
                out=pT[:, hs, :],
                in_=PS[0:S, 1024 + 520 * g:1024 + 520 * g + 4 * S]
                    .rearrange("p (a b) -> p a b", a=4))
            # yT = v^T @ p per head (swapped operands) -> [c, s] slots
            for h in range(4 * g, 4 * (g + 1)):
                nc.tensor.matmul(out=PS[0:C, 2560 + C * h:2560 + C * h + S],
                                 lhsT=v_sb[:, h, :], rhs=pT[:, h, :],
                                 start=True, stop=True)
            # transpose the gate into [c, h, s] during the same window
            for h in range(4 * g, 4 * (g + 1)):
                nc.tensor.transpose(
                    out=PS[:, LROW + NSEL * h:LROW + NSEL * (h + 1)],
                    in_=sigrd[:, h, :], identity=ident_t[0:NSEL, 0:NSEL])
            nc.scalar.copy(
                out=rap(sgT_sb[:, :, :],
                        [[H * NSEL, C], [NSEL, 4], [1, NSEL]],
                        offset=4 * g * NSEL),
                in_=PS[:, LROW + 256 * g:LROW + 256 * (g + 1)].rearrange(
                    "p (h s) -> p h s", h=4))
            yTg = rap(PS[:, :], [[4096, C], [C, 4], [1, NSEL]],
                      offset=2560 + 4 * g * C)
            nc.vector.tensor_tensor(
                out=rap(ygT[:, :, :], [[H * NSEL, C], [NSEL, 4], [1, NSEL]],
                        offset=4 * g * NSEL),
                in0=yTg,
                in1=rap(sgT_sb[:, :, :],
                        [[H * NSEL, C], [NSEL, 4], [1, NSEL]],
                        offset=4 * g * NSEL), op=ALU.mult)

        if LEVEL == 5:
            yg32 = sb.tile([NSEL, C], F32, tag="yg32")
            nc.vector.tensor_copy(out=yg32[:, :], in_=ygT[0:NSEL, 0, :])
            nc.sync.dma_start(out=out[:, :], in_=yg32[:, :])
            return

        # ---------------- output projection ----------------
        out_sb = sb.tile([NSEL, C], F32, tag="out_sb")
        for half in range(2):
            cols = slice(64 * half, 64 * (half + 1))
            out_ps = PS[0:NSEL, 64 * half:64 * (half + 1)]
            for h in range(H):
                nc.tensor.matmul(out=out_ps, lhsT=ygT[:, h, :],
                                 rhs=woTb[:, h, cols], start=(h == 0),
                                 stop=(h == H - 1))
            nc.scalar.copy(out=out_sb[:, cols], in_=out_ps)
            nc.sync.dma_start(out=out[:, cols], in_=out_sb[:, cols])


def make_host_constants(inputs):
    """Host-side prep of tables derived from the (full) inputs."""
    cos = np.asarray(inputs["cos"]).reshape(S, 64).astype(np.float32)
    sin = np.asarray(inputs["sin"]).reshape(S, 64).astype(np.float32)
    sink = np.asarray(inputs["sink"]).reshape(H, C).astype(np.float32)
    tao = np.asarray(inputs["tao"]).astype(np.float32)
    wq = np.asarray(inputs["W_qkvg"]).astype(np.float32)
    wo = np.asarray(inputs["W_out"]).astype(np.float32)

    # partition p (0..63 in each half) holds position p+1; rows duplicated
    # for the q half (0:64) and k half (64:128)
    pos = np.arange(64) + 1
    cos_p = cos[pos]
    sin_p = sin[pos]
    cosdup = np.tile(np.concatenate([cos_p, cos_p], axis=1), (2, 1))
    sinpm = np.tile(np.concatenate([sin_p, -sin_p], axis=1), (2, 1))
    # tao folds into the rope tables: qn = (qk*rf)*cos' + swap(qk*rf)*sin'
    taocol = np.concatenate([np.full((64, 1), tao[0], np.float32),
                             np.full((64, 1), tao[1], np.float32)])
    cosdup = cosdup * taocol
    sinpm = sinpm * taocol

    # additive causal mask in s-major layout (row/col 64 = sink, pos 0)
    posf = np.where(np.arange(S) < NSEL, np.arange(S) + 1, 0)
    cmaskm = np.where(posf[None, :] <= posf[:, None], 0.0,
                      NEG_BIG).astype(np.float32)
    negio = (float(NP) - np.arange(NP, dtype=np.float32)).reshape(1, NP)

    # sink rows: rope at position 0 is identity; rmsnorm + tao on host
    sn = sink / np.sqrt((sink * sink).mean(axis=-1, keepdims=True) + EPS)
    sinkTq = np.ascontiguousarray((sn * tao[0]).T)
    sinkTk = np.ascontiguousarray((sn * tao[1]).T)
    # v sink row as bf16 bit pattern (round-to-nearest-even)
    f = sink.reshape(1, H * C).astype(np.float32)
    u = f.view(np.uint32)
    rounded = ((u + 0x7FFF + ((u >> 16) & 1)) >> 16).astype(np.uint16)
    sinkvb = np.ascontiguousarray(rounded)

    # token ids: row 16T+4p+t gathers token 16*I[p] + 4T + t
    # sel16[j, r] = 16*(j==p(r)) for j<4; sel16[4, r] = 4T(r) + t(r)
    sel16m = np.zeros((5, NSEL), np.float32)
    for Tn in range(4):
        for p in range(4):
            for t in range(4):
                r = 16 * Tn + 4 * p + t
                sel16m[p, r] = 16.0
                sel16m[4, r] = float(4 * Tn + t)

    wqT = np.ascontiguousarray(wq.T)
    woT = np.ascontiguousarray(wo.reshape(C, H, C).transpose(2, 1, 0))

    ident = np.eye(128, dtype=np.float32)
    tabs = np.zeros((128, 401), np.float32)
    tabs[:, 0:128] = cosdup
    tabs[:, 128:256] = sinpm
    tabs[:128, 256:264] = sinkTq
    tabs[:128, 264:272] = sinkTk
    tabs[:S, 272:337] = cmaskm
    tabs[:5, 337:401] = sel16m
    return dict(tabs=tabs, negio=negio, sinkvb=sinkvb,
                wqT_d=wqT, woT_d=woT, ident=ident)


_CACHE = {}


def get_nc():
    if "nc" not in _CACHE:
        nc = bacc.Bacc("TRN2", target_bir_lowering=False, debug=False,
                       num_devices=B)
        build_kernel(nc)
        nc.compile()
        _CACHE["nc"] = nc
    return _CACHE["nc"]


def make_in_maps(inputs):
    x = np.ascontiguousarray(inputs["x"], dtype=np.float32)
    pwv = np.concatenate(
        [np.asarray(inputs["patch_w"], np.float32).ravel(),
         np.ones(128, np.float32)]).reshape(1, PATCH + 128)
    consts = make_host_constants(inputs)
    in_maps = []
    for b in range(B):
        m = {"xb": np.ascontiguousarray(x[b]), "pw": pwv}
        m.update(consts)
        in_maps.append(m)
    return in_maps


def kernel(**inputs):
    nc = get_nc()
    in_maps = make_in_maps(inputs)
    res = run_bass_kernel_spmd(nc, in_maps, core_ids=list(range(B)))
    return np.stack([r["out"] for r in res.results], axis=0)


if __name__ == "__main__":
    nc = get_nc()
    print("build ok:", len(nc.m.functions[0].allocations), "allocations")
